# revision 1
# baseline (speedup 1.0000x reference)
"""Trainium2 Bass kernel for nn_BlockDiagonalLinear_text (hyperbolic block-diag linear).

Math: the reference's per-row operations are all scalar row-scalings, so
  out = alpha_row * y   with  y = x @ blockdiag(W_1..W_16).T
where alpha_row is a chain of tanh/artanh/sqrt scalars of ||x_row|| and
||y_row||.  (The expmap scale s cancels out of res_c except through
saturated tanh arguments - validated numerically against the reference.)

Sharding: data-parallel over rows. 8192 rows -> 8 cores x 1024 rows.
Weights (4 MB + identity) replicated. Per-core kernel streams 8 tiles of
128 rows:
  DMA x tile -> ACT x^2 row-sums -> PE transpose x (128x128 blocks) ->
  fp32r (FP22 single-pass) block matmuls -> DVE copy y to SBUF ->
  ACT y^2 row-sums -> per-row scalar chain ([128,1] ops) ->
  DVE scale y in place -> DMA out.

Uses bacc.Bacc (not raw bass.Bass): its compile() pass legalizes
semaphore waits for the 1-wait-per-instruction TPB ISA (EVSEM splitting,
matmul-wait relocation to LDWEIGHTS).
"""
import sys
import numpy as np

for _p in ("/opt/trn_rl_repo", "/root/.axon_site/_ro/trn_rl_repo"):
    if _p not in sys.path:
        sys.path.append(_p)

import concourse.bass as bass
import concourse.bacc as bacc
import concourse.mybir as mybir
from concourse import tile
from concourse.bass_utils import run_bass_kernel_spmd

R, BS = 16, 256           # 16 diagonal blocks of 256x256
D = R * BS                # 4096
P = 128                   # partitions
N_CORES = 8
ROWS_TOTAL = 4 * 2048     # 8192
ROWS_CORE = ROWS_TOTAL // N_CORES   # 1024
NT = ROWS_CORE // P       # 8 tiles of 128 rows per core
WCOLS = 2 * R * BS        # 8192 weight columns
WIDC = WCOLS + P          # + identity columns

f32 = mybir.dt.float32
f32r = mybir.dt.float32r
AF = mybir.ActivationFunctionType
OP = mybir.AluOpType

CLIP_Z = float(np.float32(1.0) - np.float32(1e-5))          # 0.99999
MAXNORM = float(np.float32(1.0 - 1e-3) / np.float32(0.1))   # 9.99


def build_nc(ablate=()):
    ablate = set(ablate)
    nc = bacc.Bacc()
    # float32r (FP22) end-to-end on the matmul path: walrus requires every
    # producer feeding an fp32r matmul to declare an fp32r output.
    x_d = nc.declare_dram_parameter("x", [ROWS_CORE, D], f32r, isOutput=False)
    w_d = nc.declare_dram_parameter("w", [P, WIDC], f32r, isOutput=False)
    out_d = nc.declare_dram_parameter("out", [ROWS_CORE, D], f32, isOutput=True)

    with tile.TileContext(nc) as tc:
        with (
            tc.tile_pool(name="wpool", bufs=1) as wpool,
            tc.tile_pool(name="xpool", bufs=2) as xpool,
            tc.tile_pool(name="ypool", bufs=3) as ypool,
            tc.tile_pool(name="xtpool", bufs=2) as xtpool,
            tc.tile_pool(name="scrpool", bufs=1) as scrpool,
            tc.tile_pool(name="stats", bufs=2) as stats,
            tc.tile_pool(name="pst", bufs=2, space="PSUM") as pst,
            tc.tile_pool(name="psy", bufs=4, space="PSUM") as psy,
        ):
            w_sb = wpool.tile([P, WIDC], f32r, name="w_sb")
            nc.sync.dma_start(out=w_sb[:], in_=w_d[:])
            id_sb = w_sb[:, WCOLS:WIDC]
            scratch = scrpool.tile([P, D], f32, name="scratch")

            def st(shape, tag):
                return stats.tile(shape, f32, tag=tag, name=tag)

            for i in range(NT):
                x_sb = xpool.tile([P, D], f32r, tag="x", name=f"x_{i}")
                nc.sync.dma_start(out=x_sb[:], in_=x_d[i * P:(i + 1) * P, :])

                q2 = st([P, 2], "q2")
                # qx = sum_k x^2 (row-wise)
                if "squares" not in ablate:
                    nc.scalar.activation(scratch[:], x_sb[:].bitcast(f32),
                                         AF.Square, accum_out=q2[:, 0:1])

                # transpose x tile: xt[:, c*128:+128] = x[:, c*128:+128].T
                xt_sb = xtpool.tile([P, D], f32r, tag="xt", name=f"xt_{i}")
                for c in range(D // P):
                    tp = pst.tile([P, P], f32r, tag="tp", name=f"tp_{i}_{c}")
                    nc.tensor.transpose(tp[:], x_sb[:, c * P:(c + 1) * P], id_sb)
                    nc.vector.tensor_copy(xt_sb[:, c * P:(c + 1) * P], tp[:])

                # block matmuls: y[:, r*256:+256] = x_blk_r @ W_r.T  (fp32r)
                y_sb = ypool.tile([P, D], f32, tag="y", name=f"y_{i}")
                for r in range(R):
                    py = psy.tile([P, BS], f32, tag="py", name=f"py_{i}_{r}")
                    for c in range(2):
                        kc = 2 * r + c
                        nc.tensor.matmul(
                            py[:],
                            xt_sb[:, kc * P:(kc + 1) * P],
                            w_sb[:, kc * BS:(kc + 1) * BS],
                            start=(c == 0), stop=(c == 1),
                        )
                    nc.vector.tensor_copy(y_sb[:, r * BS:(r + 1) * BS], py[:])

                # qy = sum_j y^2 (row-wise)
                if "squares" not in ablate:
                    nc.scalar.activation(scratch[:], y_sb[:], AF.Square,
                                         accum_out=q2[:, 1:2])

                # ---- per-row scalar chain ([128,1] / [128,2] ops) ----
                V = nc.vector
                if "chain" in ablate:
                    alm = st([P, 1], "alm")
                    V.tensor_scalar_mul(alm[:], q2[:, 1:2], 1.0)
                    if "scale" not in ablate:
                        V.tensor_scalar(out=y_sb[:], in0=y_sb[:], scalar1=alm[:],
                                        scalar2=5.0, op0=OP.mult, op1=OP.mult)
                    nc.sync.dma_start(out=out_d[i * P:(i + 1) * P, :], in_=y_sb[:])
                    continue
                lnq = st([P, 2], "lnq")
                nc.scalar.activation(lnq[:], q2[:], AF.Ln)
                U = st([P, 2], "U")   # [u | y_n] = sqrt via exp(0.5 ln q)
                nc.scalar.activation(U[:], lnq[:], AF.Exp, scale=0.5)

                uc = st([P, 1], "uc")
                V.tensor_scalar_max(uc[:], U[:, 0:1], 1e-5)
                t1 = st([P, 1], "t1")
                V.tensor_scalar_mul(t1[:], uc[:], 0.1)
                r1 = st([P, 1], "r1")
                V.reciprocal(r1[:], t1[:])
                args_ = st([P, 1], "args_")
                V.tensor_scalar_min(args_[:], t1[:], 15.0)
                Es = st([P, 1], "Es")
                nc.scalar.activation(Es[:], args_[:], AF.Exp, scale=2.0)
                e1 = st([P, 1], "e1")
                V.tensor_scalar_add(e1[:], Es[:], 1.0)
                r2 = st([P, 1], "r2")
                V.reciprocal(r2[:], e1[:])
                tsx = st([P, 1], "tsx")   # tanh(0.1 u_c)
                V.tensor_scalar(out=tsx[:], in0=r2[:], scalar1=-2.0, scalar2=1.0,
                                op0=OP.mult, op1=OP.add)
                za = st([P, 1], "za")
                V.tensor_scalar_min(za[:], tsx[:], CLIP_Z)
                L = st([P, 2], "L")
                V.tensor_scalar_add(L[:, 0:1], za[:], 1.0)
                V.tensor_scalar(out=L[:, 1:2], in0=za[:], scalar1=-1.0, scalar2=1.0,
                                op0=OP.mult, op1=OP.add)
                lnL = st([P, 2], "lnL")
                nc.scalar.activation(lnL[:], L[:], AF.Ln)
                d_ = st([P, 1], "d_")     # 2*artanh(za)
                V.tensor_sub(d_[:], lnL[:, 0:1], lnL[:, 1:2])
                yns = st([P, 1], "yns")   # y_n clamped for safe reciprocal
                V.tensor_scalar_max(yns[:], U[:, 1:2], 1e-20)
                w1 = st([P, 1], "w1")
                V.tensor_mul(w1[:], U[:, 1:2], r1[:])
                w2 = st([P, 1], "w2")
                V.tensor_mul(w2[:], w1[:], d_[:])
                argt = st([P, 1], "argt")
                V.tensor_scalar(out=argt[:], in0=w2[:], scalar1=0.05, scalar2=15.0,
                                op0=OP.mult, op1=OP.min)
                Et = st([P, 1], "Et")
                nc.scalar.activation(Et[:], argt[:], AF.Exp, scale=2.0)
                e2 = st([P, 1], "e2")
                V.tensor_scalar_add(e2[:], Et[:], 1.0)
                r3 = st([P, 1], "r3")
                V.reciprocal(r3[:], e2[:])
                ttx = st([P, 1], "ttx")   # tanh(arg_t)
                V.tensor_scalar(out=ttx[:], in0=r3[:], scalar1=-2.0, scalar2=1.0,
                                op0=OP.mult, op1=OP.add)
                nrm = st([P, 1], "nrm")
                V.tensor_scalar(out=nrm[:], in0=ttx[:], scalar1=10.0, scalar2=1e-5,
                                op0=OP.mult, op1=OP.max)
                ryn = st([P, 1], "ryn")
                V.reciprocal(ryn[:], yns[:])
                gs = st([P, 1], "gs")
                V.tensor_mul(gs[:], ttx[:], ryn[:])
                rn = st([P, 1], "rn")
                V.reciprocal(rn[:], nrm[:])
                p9 = st([P, 1], "p9")
                V.tensor_scalar_mul(p9[:], rn[:], MAXNORM)
                pf = st([P, 1], "pf")
                V.tensor_scalar_min(pf[:], p9[:], 1.0)
                m_ = st([P, 1], "m_")
                V.tensor_scalar_min(m_[:], nrm[:], MAXNORM)
                zb = st([P, 1], "zb")
                V.tensor_scalar_mul(zb[:], m_[:], 0.1)
                B = st([P, 2], "B")
                V.tensor_scalar_add(B[:, 0:1], zb[:], 1.0)
                V.tensor_scalar(out=B[:, 1:2], in0=zb[:], scalar1=-1.0, scalar2=1.0,
                                op0=OP.mult, op1=OP.add)
                lnB = st([P, 2], "lnB")
                nc.scalar.activation(lnB[:], B[:], AF.Ln)
                db = st([P, 1], "db")     # 2*artanh(0.1 m)
                V.tensor_sub(db[:], lnB[:, 0:1], lnB[:, 1:2])
                rzb = st([P, 1], "rzb")
                V.reciprocal(rzb[:], zb[:])
                a1 = st([P, 1], "a1")
                V.tensor_mul(a1[:], gs[:], pf[:])
                a2 = st([P, 1], "a2")
                V.tensor_mul(a2[:], db[:], rzb[:])
                al = st([P, 1], "al")
                V.tensor_mul(al[:], a1[:], a2[:])
                mask = st([P, 1], "mask")
                V.tensor_scalar(out=mask[:], in0=q2[:, 1:2], scalar1=0.0, scalar2=None,
                                op0=OP.is_gt)
                alm = st([P, 1], "alm")
                V.tensor_mul(alm[:], al[:], mask[:])

                # out = y * alpha * 5  (5 = 10 from gs x 0.5 from artanh halves)
                if "scale" not in ablate:
                    V.tensor_scalar(out=y_sb[:], in0=y_sb[:], scalar1=alm[:],
                                    scalar2=5.0, op0=OP.mult, op1=OP.mult)
                nc.sync.dma_start(out=out_d[i * P:(i + 1) * P, :], in_=y_sb[:])
    nc.finalize()   # Bacc.compile(): reg alloc + EVSEM wait legalization
    return nc


_NC = None


def _get_nc():
    global _NC
    if _NC is None:
        _NC = build_nc()
    return _NC


def _round_fp22(a: np.ndarray) -> np.ndarray:
    # round-to-nearest-even to 13-bit mantissa (float32r / FP22)
    u = a.astype(np.float32).view(np.uint32)
    keep = np.uint32(0xFFFFFC00)
    low = u & np.uint32(0x3FF)
    half = np.uint32(0x200)
    lsb = (u >> np.uint32(10)) & np.uint32(1)
    round_up = (low > half) | ((low == half) & (lsb == 1))
    u = (u & keep) + (round_up.astype(np.uint32) << np.uint32(10))
    return u.view(np.float32)


def _prep_weights(weights: np.ndarray) -> np.ndarray:
    # w_sb[:, (2r+c)*256:+256][p, j] = W[r, j, k=c*128+p]; identity appended.
    wt = (weights.astype(np.float32).transpose(0, 2, 1)      # [r, k, j]
          .reshape(R, 2, P, BS).transpose(2, 0, 1, 3)        # [p, r, c, j]
          .reshape(P, WCOLS))
    return np.ascontiguousarray(
        np.concatenate([_round_fp22(wt), np.eye(P, dtype=np.float32)], axis=1))


def kernel(x: np.ndarray, weights: np.ndarray) -> np.ndarray:
    nc = _get_nc()
    xf = np.ascontiguousarray(x, dtype=np.float32).reshape(ROWS_TOTAL, D)
    wid = _prep_weights(np.asarray(weights))
    in_maps = [
        {"x": xf[i * ROWS_CORE:(i + 1) * ROWS_CORE], "w": wid}
        for i in range(N_CORES)
    ]
    res = run_bass_kernel_spmd(nc, in_maps, list(range(N_CORES)))
    out = np.concatenate([res.results[i]["out"] for i in range(N_CORES)], axis=0)
    return out.reshape(x.shape).astype(np.float32, copy=False)


if __name__ == "__main__":
    xs = np.random.randn(4, 2048, D).astype(np.float32)
    ws = (np.broadcast_to(np.eye(BS, dtype=np.float32), (R, BS, BS))
          + 0.02 * np.random.randn(R, BS, BS).astype(np.float32))
    o = kernel(xs, ws)
    print("kernel ran, out shape", o.shape, o.dtype)



# revision 4
# speedup vs baseline: 2.1674x; 2.1674x over previous
"""Trainium2 Bass kernel for nn_BlockDiagonalLinear_text (hyperbolic block-diag linear).

Math: every per-row operation in the reference is a scalar row-scaling of
  y = x @ blockdiag(W_1..W_16).T
and the scalar chain collapses via artanh(tanh(t)) = t:
  out = 10 * clamp(y_n * k1, 1e-6, CB) / y_n * y     (y_n = ||y||)
  k1  = min(0.1*uc, CA) / uc,  uc = max(||x||, 1e-5)
  CA  = artanh(f32(1 - 1e-5))   (expmap tanh always lands in the artanh clip)
  CB  = artanh(f32(0.1) * f32(9.99))  (the _project maxnorm cancels pf*ttx)
k1 depends only on x, so it is precomputed on the host (one fused scalar
per row) alongside the input layout transform.

Device layout (per core, data-parallel over rows: 8192 rows -> 8 x 1024):
  xt  [128, 8*32*128] bf16 -- x pre-transposed on host so each matmul's
      stationary operand xt[:, i*4096 + kc*128 :+128] = x[tile rows, k-chunk].T
      DMAs at full rate (8 KiB/partition/tile contiguous), no PE transposes,
      no PSUM->SBUF cast copies.
  w   [128, 8192] bf16 -- w[p, kc*256+j] = W[kc//2, j, (kc%2)*128+p]
  k1  [128, 8] f32 per-tile row scalars
Per 128-row tile: 32 bf16 matmuls (2 per 256-col block) into 8 PSUM banks
[128,512]; ACT Square+accum per bank -> qy partials; DVE copies bank->SBUF;
tiny DVE chain -> alm; DVE scaled in-place; DMA out f32.
Single ACT table (Square/Rsqrt/Copy) -> zero ACT_TABLE_LOADs.
"""
import sys
import numpy as np

for _p in ("/opt/trn_rl_repo", "/root/.axon_site/_ro/trn_rl_repo"):
    if _p not in sys.path:
        sys.path.append(_p)

import ml_dtypes
import concourse.bass as bass
import concourse.bacc as bacc
import concourse.mybir as mybir
from concourse import tile
from concourse.bass_utils import run_bass_kernel_spmd

R, BS = 16, 256           # 16 diagonal blocks of 256x256
D = R * BS                # 4096
P = 128                   # partitions
N_CORES = 8
ROWS_TOTAL = 4 * 2048     # 8192
ROWS_CORE = ROWS_TOTAL // N_CORES   # 1024
NT = ROWS_CORE // P       # 8 tiles of 128 rows per core
NKC = D // P              # 32 k-chunks of 128
NB = 8                    # PSUM banks per tile (512 cols each)
BANK = 512

f32 = mybir.dt.float32
bf16 = mybir.dt.bfloat16
AF = mybir.ActivationFunctionType
OP = mybir.AluOpType
AX = None  # set lazily (bass_rust import)

CA = 6.10235526389634     # artanh(f32(1 - 1e-5))
CB = 3.800207607813536    # artanh(f32(0.1) * f32((1-1e-3)/0.1))


def build_nc():
    import bass_rust
    nc = bacc.Bacc()
    xt_d = nc.declare_dram_parameter("xt", [P, NT * D], bf16, isOutput=False)
    w_d = nc.declare_dram_parameter("w", [P, 2 * R * BS], bf16, isOutput=False)
    k1_d = nc.declare_dram_parameter("k1", [P, NT], f32, isOutput=False)
    out_d = nc.declare_dram_parameter("out", [ROWS_CORE, D], f32, isOutput=True)

    with tile.TileContext(nc) as tc:
        with (
            tc.tile_pool(name="xtpool", bufs=1) as xtpool,
            tc.tile_pool(name="wpool", bufs=1) as wpool,
            tc.tile_pool(name="kpool", bufs=1) as kpool,
            tc.tile_pool(name="ypool", bufs=3) as ypool,
            tc.tile_pool(name="scrpool", bufs=1) as scrpool,
            tc.tile_pool(name="stats", bufs=4) as stats,
            tc.tile_pool(name="pst", bufs=8, space="PSUM") as pst,
        ):
            k1_sb = kpool.tile([P, NT], f32, name="k1_sb")
            nc.sync.dma_start(out=k1_sb[:], in_=k1_d[:])

            # xt tile 0 first (first matmul gate), then weights, then the rest
            xts = [None] * NT
            xts[0] = xtpool.tile([P, D], bf16, tag="xt0", name="xt_0")
            nc.sync.dma_start(out=xts[0][:], in_=xt_d[:, 0:D])
            wts = []
            for g in range(4):
                wt = wpool.tile([P, 8 * BS], bf16, tag=f"w{g}", name=f"w_{g}")
                nc.sync.dma_start(out=wt[:], in_=w_d[:, g * 8 * BS:(g + 1) * 8 * BS])
                wts.append(wt)
            for i in range(1, NT):
                xts[i] = xtpool.tile([P, D], bf16, tag=f"xt{i}", name=f"xt_{i}")
                nc.sync.dma_start(out=xts[i][:], in_=xt_d[:, i * D:(i + 1) * D])

            scr = scrpool.tile([P, BANK], f32, name="scr")

            def st(shape, tag):
                return stats.tile(shape, f32, tag=tag, name=tag)

            V = nc.vector
            for i in range(NT):
                y_sb = ypool.tile([P, D], f32, tag="y", name=f"y_{i}")
                qyp = st([P, NB], f"qyp")
                for b in range(NB):
                    py = pst.tile([P, BANK], f32, tag="py", name=f"py_{i}_{b}")
                    for blk in range(2):
                        r = 2 * b + blk
                        for c in range(2):
                            kc = 2 * r + c
                            nc.tensor.matmul(
                                py[:, blk * BS:(blk + 1) * BS],
                                xts[i][:, kc * P:(kc + 1) * P],
                                wts[kc // 8][:, (kc % 8) * BS:(kc % 8 + 1) * BS],
                                start=(c == 0), stop=(c == 1),
                            )
                    # qy partial on ACT; y bank copy on DVE
                    nc.scalar.activation(scr[:], py[:], AF.Square,
                                         accum_out=qyp[:, b:b + 1])
                    V.tensor_copy(y_sb[:, b * BANK:(b + 1) * BANK], py[:])

                # ---- collapsed per-row chain ----
                qy = st([P, 1], "qy")
                V.reduce_sum(qy[:], qyp[:], axis=bass_rust.AxisListType.X)
                qyc = st([P, 1], "qyc")
                V.tensor_scalar_max(qyc[:], qy[:], 1e-38)
                y_n = st([P, 1], "y_n")
                nc.scalar.activation(y_n[:], qyc[:], AF.Sqrt)
                ry = st([P, 1], "ry")
                V.reciprocal(ry[:], y_n[:])
                w2 = st([P, 1], "w2")
                V.tensor_tensor(w2[:], y_n[:], k1_sb[:, i:i + 1], OP.mult)
                g_ = st([P, 1], "g_")
                V.tensor_scalar(out=g_[:], in0=w2[:], scalar1=1e-6, scalar2=CB,
                                op0=OP.max, op1=OP.min)
                a_ = st([P, 1], "a_")
                V.tensor_tensor(a_[:], g_[:], ry[:], OP.mult)
                alm = st([P, 1], "alm")
                V.scalar_tensor_tensor(out=alm[:], in0=qy[:], scalar=0.0,
                                       in1=a_[:], op0=OP.is_gt, op1=OP.mult)
                V.tensor_scalar(out=y_sb[:], in0=y_sb[:], scalar1=alm[:],
                                scalar2=10.0, op0=OP.mult, op1=OP.mult)
                nc.sync.dma_start(out=out_d[i * P:(i + 1) * P, :], in_=y_sb[:])
    nc.finalize()
    return nc


_NC = None


def _get_nc():
    global _NC
    if _NC is None:
        _NC = build_nc()
    return _NC


def _prep_inputs(x: np.ndarray, weights: np.ndarray):
    xf = np.ascontiguousarray(x, dtype=np.float32).reshape(ROWS_TOTAL, D)
    # w[p, kc*256+j] = W[kc//2, j, (kc%2)*128+p]
    wt = (weights.astype(np.float32).transpose(0, 2, 1)   # [r, k, j]
          .reshape(R, 2, P, BS).transpose(2, 0, 1, 3)     # [p, r, c, j]
          .reshape(P, 2 * R * BS)).astype(ml_dtypes.bfloat16)
    wt = np.ascontiguousarray(wt)

    qx = np.einsum('ij,ij->i', xf.astype(np.float64), xf.astype(np.float64))
    uc = np.maximum(np.sqrt(qx), 1e-5)
    k1 = (np.minimum(0.1 * uc, CA) / uc).astype(np.float32)

    in_maps = []
    for cidx in range(N_CORES):
        xc = xf[cidx * ROWS_CORE:(cidx + 1) * ROWS_CORE]
        # xt[p, ((i*32 + kc)*128) + r] = xc[i*128 + r, kc*128 + p]
        xt = (xc.reshape(NT, P, NKC, P).transpose(3, 0, 2, 1)
              .reshape(P, NT * D)).astype(ml_dtypes.bfloat16)
        k1c = np.ascontiguousarray(
            k1[cidx * ROWS_CORE:(cidx + 1) * ROWS_CORE].reshape(NT, P).T)
        in_maps.append({
            "xt": np.ascontiguousarray(xt),
            "w": wt,
            "k1": k1c,
        })
    return in_maps


def kernel(x: np.ndarray, weights: np.ndarray) -> np.ndarray:
    nc = _get_nc()
    in_maps = _prep_inputs(x, np.asarray(weights))
    res = run_bass_kernel_spmd(nc, in_maps, list(range(N_CORES)))
    out = np.concatenate([res.results[i]["out"] for i in range(N_CORES)], axis=0)
    return out.reshape(x.shape).astype(np.float32, copy=False)


if __name__ == "__main__":
    xs = np.random.randn(4, 2048, D).astype(np.float32)
    ws = (np.broadcast_to(np.eye(BS, dtype=np.float32), (R, BS, BS))
          + 0.02 * np.random.randn(R, BS, BS).astype(np.float32))
    o = kernel(xs, ws)
    print("kernel ran, out shape", o.shape, o.dtype)


# revision 7
# speedup vs baseline: 2.3645x; 1.0909x over previous
"""Trainium2 Bass kernel for nn_BlockDiagonalLinear_text (hyperbolic block-diag linear).

Math: every per-row operation in the reference is a scalar row-scaling of
  y = x @ blockdiag(W_1..W_16).T
and the scalar chain collapses via artanh(tanh(t)) = t:
  out = 10 * clamp(y_n * k1, 1e-6, CB) / y_n * y     (y_n = ||y||)
  k1  = min(0.1*uc, CA) / uc,  uc = max(||x||, 1e-5)
  CA  = artanh(f32(1 - 1e-5))   (expmap tanh always lands in the artanh clip)
  CB  = artanh(f32(0.1) * f32(9.99))  (the _project maxnorm cancels pf*ttx)
k1 depends only on x, so it is precomputed on the host (one fused scalar
per row) alongside the input layout transform.

Device layout (per core, data-parallel over rows: 8192 rows -> 8 x 1024):
  xt  [128, 8*32*128] bf16 -- x pre-transposed on host so each matmul's
      stationary operand xt[:, i*4096 + kc*128 :+128] = x[tile rows, k-chunk].T
      DMAs at full rate (8 KiB/partition/tile contiguous), no PE transposes,
      no PSUM->SBUF cast copies.
  w   [128, 8192] bf16 -- w[p, kc*256+j] = W[kc//2, j, (kc%2)*128+p]
  k1  [128, 8] f32 per-tile row scalars
Per 128-row tile: 32 bf16 matmuls (2 per 256-col block) into 8 PSUM banks
[128,512]; ACT Square+accum per bank -> qy partials; DVE copies bank->SBUF;
tiny DVE chain -> alm; DVE scaled in-place; DMA out f32.
Single ACT table (Square/Rsqrt/Copy) -> zero ACT_TABLE_LOADs.
"""
import sys
import numpy as np

for _p in ("/opt/trn_rl_repo", "/root/.axon_site/_ro/trn_rl_repo"):
    if _p not in sys.path:
        sys.path.append(_p)

import ml_dtypes
import concourse.bass as bass
import concourse.bacc as bacc
import concourse.mybir as mybir
from concourse import tile
from concourse.bass_utils import run_bass_kernel_spmd

R, BS = 16, 256           # 16 diagonal blocks of 256x256
D = R * BS                # 4096
P = 128                   # partitions
N_CORES = 8
ROWS_TOTAL = 4 * 2048     # 8192
ROWS_CORE = ROWS_TOTAL // N_CORES   # 1024
NT = ROWS_CORE // P       # 8 tiles of 128 rows per core
NKC = D // P              # 32 k-chunks of 128
NB = 8                    # PSUM banks per tile (512 cols each)
BANK = 512

f32 = mybir.dt.float32
bf16 = mybir.dt.bfloat16
AF = mybir.ActivationFunctionType
OP = mybir.AluOpType
AX = None  # set lazily (bass_rust import)

CA = 6.10235526389634     # artanh(f32(1 - 1e-5))
CB = 3.800207607813536    # artanh(f32(0.1) * f32((1-1e-3)/0.1))


def build_nc():
    import bass_rust
    nc = bacc.Bacc()
    xt_d = nc.declare_dram_parameter("xt", [P, NT * D], bf16, isOutput=False)
    w_d = nc.declare_dram_parameter("w", [P, 2 * R * BS], bf16, isOutput=False)
    k1_d = nc.declare_dram_parameter("k1", [P, NT], f32, isOutput=False)
    out_d = nc.declare_dram_parameter("out", [ROWS_CORE, D], f32, isOutput=True)

    with tile.TileContext(nc) as tc:
        with (
            tc.tile_pool(name="xtpool", bufs=1) as xtpool,
            tc.tile_pool(name="wpool", bufs=1) as wpool,
            tc.tile_pool(name="kpool", bufs=1) as kpool,
            tc.tile_pool(name="ypool", bufs=4) as ypool,
            tc.tile_pool(name="scrpool", bufs=1) as scrpool,
            tc.tile_pool(name="stats", bufs=4) as stats,
            tc.tile_pool(name="pst", bufs=8, space="PSUM") as pst,
        ):
            k1_sb = kpool.tile([P, NT], f32, name="k1_sb")
            nc.sync.dma_start(out=k1_sb[:], in_=k1_d[:])

            # xt tile 0 first (first matmul gate), then weights, then the rest
            xts = [None] * NT
            xts[0] = xtpool.tile([P, D], bf16, tag="xt0", name="xt_0")
            nc.sync.dma_start(out=xts[0][:], in_=xt_d[:, 0:D])
            wts = []
            for g in range(4):
                wt = wpool.tile([P, 8 * BS], bf16, tag=f"w{g}", name=f"w_{g}")
                nc.sync.dma_start(out=wt[:], in_=w_d[:, g * 8 * BS:(g + 1) * 8 * BS])
                wts.append(wt)
            for i in range(1, NT):
                xts[i] = xtpool.tile([P, D], bf16, tag=f"xt{i}", name=f"xt_{i}")
                nc.sync.dma_start(out=xts[i][:], in_=xt_d[:, i * D:(i + 1) * D])

            scr = scrpool.tile([P, D], f32, name="scr")

            def st(shape, tag):
                return stats.tile(shape, f32, tag=tag, name=tag)

            V = nc.vector
            for i in range(NT):
                y_sb = ypool.tile([P, D], f32, tag="y", name=f"y_{i}")
                for b in range(NB):
                    py = pst.tile([P, BANK], f32, tag="py", name=f"py_{i}_{b}")
                    for blk in range(2):
                        r = 2 * b + blk
                        for c in range(2):
                            kc = 2 * r + c
                            nc.tensor.matmul(
                                py[:, blk * BS:(blk + 1) * BS],
                                xts[i][:, kc * P:(kc + 1) * P],
                                wts[kc // 8][:, (kc % 8) * BS:(kc % 8 + 1) * BS],
                                start=(c == 0), stop=(c == 1),
                            )
                    # y bank copy: split across ACT (Copy) and DVE
                    if b < 4:
                        nc.scalar.activation(y_sb[:, b * BANK:(b + 1) * BANK],
                                             py[:], AF.Copy)
                    else:
                        V.tensor_copy(y_sb[:, b * BANK:(b + 1) * BANK], py[:])

                # ---- collapsed per-row chain ----
                # qy = sum(y^2) in one ACT pass (single accumulator read)
                qy = st([P, 1], "qy")
                nc.scalar.activation(scr[:], y_sb[:], AF.Square,
                                     accum_out=qy[:])
                qyc = st([P, 1], "qyc")
                V.tensor_scalar_max(qyc[:], qy[:], 1e-38)
                y_n = st([P, 1], "y_n")
                nc.scalar.activation(y_n[:], qyc[:], AF.Sqrt)
                ry = st([P, 1], "ry")
                V.reciprocal(ry[:], y_n[:])
                w2 = st([P, 1], "w2")
                V.tensor_tensor(w2[:], y_n[:], k1_sb[:, i:i + 1], OP.mult)
                g_ = st([P, 1], "g_")
                V.tensor_scalar(out=g_[:], in0=w2[:], scalar1=1e-6, scalar2=CB,
                                op0=OP.max, op1=OP.min)
                a_ = st([P, 1], "a_")
                V.tensor_tensor(a_[:], g_[:], ry[:], OP.mult)
                alm = st([P, 1], "alm")
                V.scalar_tensor_tensor(out=alm[:], in0=qy[:], scalar=0.0,
                                       in1=a_[:], op0=OP.is_gt, op1=OP.mult)
                V.tensor_scalar(out=y_sb[:], in0=y_sb[:], scalar1=alm[:],
                                scalar2=10.0, op0=OP.mult, op1=OP.mult)
                nc.sync.dma_start(out=out_d[i * P:(i + 1) * P, :], in_=y_sb[:])
    nc.finalize()
    return nc


_NC = None


def _get_nc():
    global _NC
    if _NC is None:
        _NC = build_nc()
    return _NC


def _prep_inputs(x: np.ndarray, weights: np.ndarray):
    xf = np.ascontiguousarray(x, dtype=np.float32).reshape(ROWS_TOTAL, D)
    # w[p, kc*256+j] = W[kc//2, j, (kc%2)*128+p]
    wt = (weights.astype(np.float32).transpose(0, 2, 1)   # [r, k, j]
          .reshape(R, 2, P, BS).transpose(2, 0, 1, 3)     # [p, r, c, j]
          .reshape(P, 2 * R * BS)).astype(ml_dtypes.bfloat16)
    wt = np.ascontiguousarray(wt)

    qx = np.einsum('ij,ij->i', xf.astype(np.float64), xf.astype(np.float64))
    uc = np.maximum(np.sqrt(qx), 1e-5)
    k1 = (np.minimum(0.1 * uc, CA) / uc).astype(np.float32)

    in_maps = []
    for cidx in range(N_CORES):
        xc = xf[cidx * ROWS_CORE:(cidx + 1) * ROWS_CORE]
        # xt[p, ((i*32 + kc)*128) + r] = xc[i*128 + r, kc*128 + p]
        xt = (xc.reshape(NT, P, NKC, P).transpose(3, 0, 2, 1)
              .reshape(P, NT * D)).astype(ml_dtypes.bfloat16)
        k1c = np.ascontiguousarray(
            k1[cidx * ROWS_CORE:(cidx + 1) * ROWS_CORE].reshape(NT, P).T)
        in_maps.append({
            "xt": np.ascontiguousarray(xt),
            "w": wt,
            "k1": k1c,
        })
    return in_maps


def kernel(x: np.ndarray, weights: np.ndarray) -> np.ndarray:
    nc = _get_nc()
    in_maps = _prep_inputs(x, np.asarray(weights))
    res = run_bass_kernel_spmd(nc, in_maps, list(range(N_CORES)))
    out = np.concatenate([res.results[i]["out"] for i in range(N_CORES)], axis=0)
    return out.reshape(x.shape).astype(np.float32, copy=False)


if __name__ == "__main__":
    xs = np.random.randn(4, 2048, D).astype(np.float32)
    ws = (np.broadcast_to(np.eye(BS, dtype=np.float32), (R, BS, BS))
          + 0.02 * np.random.randn(R, BS, BS).astype(np.float32))
    o = kernel(xs, ws)
    print("kernel ran, out shape", o.shape, o.dtype)


# revision 11
# speedup vs baseline: 2.4173x; 1.0223x over previous
"""Trainium2 Bass kernel for nn_BlockDiagonalLinear_text (hyperbolic block-diag linear).

Math: every per-row operation in the reference is a scalar row-scaling of
  y = x @ blockdiag(W_1..W_16).T
and the scalar chain collapses via artanh(tanh(t)) = t:
  out = 10 * clamp(y_n * k1, 1e-6, CB) / y_n * y     (y_n = ||y||)
  k1  = min(0.1*uc, CA) / uc,  uc = max(||x||, 1e-5)
  CA  = artanh(f32(1 - 1e-5))   (expmap tanh always lands in the artanh clip)
  CB  = artanh(f32(0.1) * f32(9.99))  (the _project maxnorm cancels pf*ttx)
k1 depends only on x, so it is precomputed on the host (one fused scalar
per row) alongside the input layout transform.

Device layout (per core, data-parallel over rows: 8192 rows -> 8 x 1024):
  xt  [128, 8*32*128] bf16 -- x pre-transposed on host so each matmul's
      stationary operand xt[:, i*4096 + kc*128 :+128] = x[tile rows, k-chunk].T
      DMAs at full rate (8 KiB/partition/tile contiguous), no PE transposes,
      no PSUM->SBUF cast copies.
  w   [128, 8192] bf16 -- w[p, kc*256+j] = W[kc//2, j, (kc%2)*128+p]
  k1  [128, 8] f32 per-tile row scalars
Per 128-row tile: 32 bf16 matmuls (2 per 256-col block) into 8 PSUM banks
[128,512]; ACT Square+accum per bank -> qy partials; DVE copies bank->SBUF;
tiny DVE chain -> alm; DVE scaled in-place; DMA out f32.
Single ACT table (Square/Rsqrt/Copy) -> zero ACT_TABLE_LOADs.
"""
import sys
import numpy as np

for _p in ("/opt/trn_rl_repo", "/root/.axon_site/_ro/trn_rl_repo"):
    if _p not in sys.path:
        sys.path.append(_p)

import ml_dtypes
import concourse.bass as bass
import concourse.bacc as bacc
import concourse.mybir as mybir
from concourse import tile
from concourse.bass_utils import run_bass_kernel_spmd

R, BS = 16, 256           # 16 diagonal blocks of 256x256
D = R * BS                # 4096
P = 128                   # partitions
N_CORES = 8
ROWS_TOTAL = 4 * 2048     # 8192
ROWS_CORE = ROWS_TOTAL // N_CORES   # 1024
NT = ROWS_CORE // P       # 8 tiles of 128 rows per core
NKC = D // P              # 32 k-chunks of 128
NB = 8                    # PSUM banks per tile (512 cols each)
BANK = 512

f32 = mybir.dt.float32
bf16 = mybir.dt.bfloat16
AF = mybir.ActivationFunctionType
OP = mybir.AluOpType
AX = None  # set lazily (bass_rust import)

CA = 6.10235526389634     # artanh(f32(1 - 1e-5))
CB = 3.800207607813536    # artanh(f32(0.1) * f32((1-1e-3)/0.1))


def build_nc():
    import bass_rust
    nc = bacc.Bacc()
    xt_d = nc.declare_dram_parameter("xt", [P, NT * D], bf16, isOutput=False)
    w_d = nc.declare_dram_parameter("w", [P, 2 * R * BS], bf16, isOutput=False)
    k1_d = nc.declare_dram_parameter("k1", [P, NT], f32, isOutput=False)
    out_d = nc.declare_dram_parameter("out", [ROWS_CORE, D], f32, isOutput=True)

    with tile.TileContext(nc) as tc:
        with (
            tc.tile_pool(name="xtpool", bufs=1) as xtpool,
            tc.tile_pool(name="wpool", bufs=1) as wpool,
            tc.tile_pool(name="kpool", bufs=1) as kpool,
            tc.tile_pool(name="ypool", bufs=4) as ypool,
            tc.tile_pool(name="scrpool", bufs=1) as scrpool,
            tc.tile_pool(name="stats", bufs=4) as stats,
            tc.tile_pool(name="pst", bufs=4, space="PSUM") as pst,
        ):
            k1_sb = kpool.tile([P, NT], f32, name="k1_sb")
            nc.sync.dma_start(out=k1_sb[:], in_=k1_d[:])

            # xt tile 0 first (first matmul gate), then weights, then the rest
            xts = [None] * NT
            xts[0] = xtpool.tile([P, D], bf16, tag="xt0", name="xt_0")
            nc.sync.dma_start(out=xts[0][:], in_=xt_d[:, 0:D])
            wts = []
            for g in range(4):
                wt = wpool.tile([P, 8 * BS], bf16, tag=f"w{g}", name=f"w_{g}")
                nc.sync.dma_start(out=wt[:], in_=w_d[:, g * 8 * BS:(g + 1) * 8 * BS])
                wts.append(wt)
            for i in range(1, NT):
                xts[i] = xtpool.tile([P, D], bf16, tag=f"xt{i}", name=f"xt_{i}")
                nc.sync.dma_start(out=xts[i][:], in_=xt_d[:, i * D:(i + 1) * D])

            scr = scrpool.tile([P, D], f32, name="scr")

            def st(shape, tag):
                return stats.tile(shape, f32, tag=tag, name=tag)

            V = nc.vector
            for i in range(NT):
                y_sb = ypool.tile([P, D], f32, tag="y", name=f"y_{i}")
                for bb in range(NB // 2):
                    # 2-bank PSUM tile: 4 blocks, 8 matmuls
                    py = pst.tile([P, 2 * BANK], f32, tag="py", name=f"py_{i}_{bb}")
                    for blk in range(4):
                        r = 4 * bb + blk
                        for c in range(2):
                            kc = 2 * r + c
                            nc.tensor.matmul(
                                py[:, blk * BS:(blk + 1) * BS],
                                xts[i][:, kc * P:(kc + 1) * P],
                                wts[kc // 8][:, (kc % 8) * BS:(kc % 8 + 1) * BS],
                                start=(c == 0), stop=(c == 1),
                            )
                    # y copy (2 banks at once): split across ACT (Copy) and DVE
                    if bb < 2:
                        nc.scalar.activation(y_sb[:, bb * 2 * BANK:(bb + 1) * 2 * BANK],
                                             py[:], AF.Copy)
                    else:
                        V.tensor_copy(y_sb[:, bb * 2 * BANK:(bb + 1) * 2 * BANK], py[:])

                # ---- collapsed per-row chain ----
                # qy = sum(y^2) in one ACT pass (single accumulator read)
                qy = st([P, 1], "qy")
                nc.scalar.activation(scr[:], y_sb[:], AF.Square,
                                     accum_out=qy[:])
                qyc = st([P, 1], "qyc")
                V.tensor_scalar_max(qyc[:], qy[:], 1e-38)
                y_n = st([P, 1], "y_n")
                nc.scalar.activation(y_n[:], qyc[:], AF.Sqrt)
                ry = st([P, 1], "ry")
                V.reciprocal(ry[:], y_n[:])
                w2 = st([P, 1], "w2")
                V.tensor_tensor(w2[:], y_n[:], k1_sb[:, i:i + 1], OP.mult)
                g_ = st([P, 1], "g_")
                V.tensor_scalar(out=g_[:], in0=w2[:], scalar1=1e-6, scalar2=CB,
                                op0=OP.max, op1=OP.min)
                a_ = st([P, 1], "a_")
                V.tensor_tensor(a_[:], g_[:], ry[:], OP.mult)
                alm = st([P, 1], "alm")
                V.scalar_tensor_tensor(out=alm[:], in0=qy[:], scalar=0.0,
                                       in1=a_[:], op0=OP.is_gt, op1=OP.mult)
                V.tensor_scalar(out=y_sb[:], in0=y_sb[:], scalar1=alm[:],
                                scalar2=10.0, op0=OP.mult, op1=OP.mult)
                # out-DMA issued from the idle Pool queue so its wait on the
                # DVE scale doesn't serialize the Sync queue
                nc.gpsimd.dma_start(out=out_d[i * P:(i + 1) * P, :], in_=y_sb[:])
    nc.finalize()
    return nc


_NC = None


def _get_nc():
    global _NC
    if _NC is None:
        _NC = build_nc()
    return _NC


def _prep_inputs(x: np.ndarray, weights: np.ndarray):
    xf = np.ascontiguousarray(x, dtype=np.float32).reshape(ROWS_TOTAL, D)
    # w[p, kc*256+j] = W[kc//2, j, (kc%2)*128+p]
    wt = (weights.astype(np.float32).transpose(0, 2, 1)   # [r, k, j]
          .reshape(R, 2, P, BS).transpose(2, 0, 1, 3)     # [p, r, c, j]
          .reshape(P, 2 * R * BS)).astype(ml_dtypes.bfloat16)
    wt = np.ascontiguousarray(wt)

    qx = np.einsum('ij,ij->i', xf.astype(np.float64), xf.astype(np.float64))
    uc = np.maximum(np.sqrt(qx), 1e-5)
    k1 = (np.minimum(0.1 * uc, CA) / uc).astype(np.float32)

    in_maps = []
    for cidx in range(N_CORES):
        xc = xf[cidx * ROWS_CORE:(cidx + 1) * ROWS_CORE]
        # xt[p, ((i*32 + kc)*128) + r] = xc[i*128 + r, kc*128 + p]
        xt = (xc.reshape(NT, P, NKC, P).transpose(3, 0, 2, 1)
              .reshape(P, NT * D)).astype(ml_dtypes.bfloat16)
        k1c = np.ascontiguousarray(
            k1[cidx * ROWS_CORE:(cidx + 1) * ROWS_CORE].reshape(NT, P).T)
        in_maps.append({
            "xt": np.ascontiguousarray(xt),
            "w": wt,
            "k1": k1c,
        })
    return in_maps


def kernel(x: np.ndarray, weights: np.ndarray) -> np.ndarray:
    nc = _get_nc()
    in_maps = _prep_inputs(x, np.asarray(weights))
    res = run_bass_kernel_spmd(nc, in_maps, list(range(N_CORES)))
    out = np.concatenate([res.results[i]["out"] for i in range(N_CORES)], axis=0)
    return out.reshape(x.shape).astype(np.float32, copy=False)


if __name__ == "__main__":
    xs = np.random.randn(4, 2048, D).astype(np.float32)
    ws = (np.broadcast_to(np.eye(BS, dtype=np.float32), (R, BS, BS))
          + 0.02 * np.random.randn(R, BS, BS).astype(np.float32))
    o = kernel(xs, ws)
    print("kernel ran, out shape", o.shape, o.dtype)
